# revision 17
# baseline (speedup 1.0000x reference)
"""Trainium2 Bass kernel for the bipartite GNN message-passing encoder.

Math:
  Hu = relu(sum_r An_r @ W_items_r^T); Hv = relu(sum_r An_r^T @ W_users_r^T)
  U  = relu(Hu @ dense_W^T + relu(uf @ uw1^T + ub1) @ uw2^T); V analogous.
  An_r = diag(cu) (adj==r) diag(cv),  cu=1/sqrt(Nu), cv=1/sqrt(Nv).

Strategy (v4, collective-free): the 8 cores partition the OUTPUT rows
(each core owns 512 items and 512 users, disjoint across the fleet) and
every core contracts over the FULL other side, so no cross-core reduction
is ever needed:
  - per-core MM work is unchanged vs. row-sharding (outputs shrink 4x/2x
    while the contraction grows 4x/2x); the msg weights are replicated
    (fp8, ~10.5MB/core) which trades cheap, fully-overlapped DMA for the
    two ReduceScatters that previously serialized into a ~50us tail.
  - degrees/cu/cv on the host; weights pre-scaled by the inner degree
    factor and ALPHA, quantized to fp8-e4m3, pre-interleaved for
    DoubleRow (2x fp8) matmuls.
  - rating masks are binary one-hots: item side + user r=1,2 built on DVE
    (is_equal over [128, 4096]-elem quarters of a resident int8 adj
    column-block), user r=3,4,5 shipped pre-built from the host to keep
    DVE comfortably under the PE stream.
  - pass-2 reads Hv/Hu straight from PSUM (no DRAM round-trip): relu on
    ACT, outer degree scale via broadcasted free-dim multiply on DVE,
    3 accumulating matmuls per side in the transposed [OUT, n] layout.
  - single 8-bank PSUM rotation: Pv0 Pv1 Pu0 Pu1 | pf pf2 pb pb2 | po_v
    po_u (prep recycles late, outputs recycle the MM banks after their
    final reads).
"""

import sys

import numpy as np

if "/opt/trn_rl_repo" not in sys.path:
    sys.path.insert(0, "/opt/trn_rl_repo")

import ml_dtypes  # noqa: E402

import concourse.bacc as bacc  # noqa: E402
import concourse.mybir as mybir  # noqa: E402
import concourse.tile as tile  # noqa: E402

FP = mybir.dt.float32
BF = mybir.dt.bfloat16
F8 = mybir.dt.float8e4
I8 = mybir.dt.int8

NU = NI = 4000
R = 5
M = 256
OUT = 75
SIDE = 64
FDIM = 128

GA, GB = 4, 2
BU = NU // GA  # 1000
BI = NI // GB  # 2000
NP = 4096  # padded contraction length (users or items)
KP = NP // 256  # 16 DoubleRow k-pairs
QV = 512  # items owned per core
QU = 512  # users owned per core
NCORES = GA * GB
ALPHA = 512.0
R_DVE_USER = 0  # user-side ratings built on DVE; the rest shipped
R_SHIP = R - R_DVE_USER

AF = mybir.ActivationFunctionType
ALU = mybir.AluOpType
PM = mybir.MatmulPerfMode


def build_program():
    from contextlib import ExitStack

    nc = bacc.Bacc("TRN2", target_bir_lowering=False, debug=False, num_devices=NCORES)

    # adjc: adj column-block for owned items, [user(kp,s,p) -> part, free]
    adjc = nc.dram_tensor("adjc", [128, KP * 2 * QV], I8, kind="ExternalInput")
    # adjt: adjT column-block for owned users (item-contraction layout)
    adjt = nc.dram_tensor("adjt", [128, KP * 2 * QU], I8, kind="ExternalInput")
    # shipped user-side one-hots for r = R_DVE_USER+1 .. R
    umask8 = nc.dram_tensor("umask8", [R_SHIP, 128, KP * 2 * QU], F8,
                            kind="ExternalInput")
    # pre-built item mask for (r=1, kp-half 0): unblocks the very first matmul
    imask0 = nc.dram_tensor("imask0", [128, 2 * (KP * 2 * QV) // 4], F8,
                            kind="ExternalInput")
    wu8 = nc.dram_tensor("wu8", [KP, 128, 2, R, M], F8, kind="ExternalInput")
    wi8 = nc.dram_tensor("wi8", [KP, 128, 2, R, M], F8, kind="ExternalInput")
    vfTq = nc.dram_tensor("vfTq", [FDIM, QV], BF, kind="ExternalInput")
    ufTq = nc.dram_tensor("ufTq", [FDIM, QU], BF, kind="ExternalInput")
    dwT = nc.dram_tensor("dwT", [2, 128, OUT], BF, kind="ExternalInput")
    uw1T = nc.dram_tensor("uw1T", [FDIM, SIDE], BF, kind="ExternalInput")
    vw1T = nc.dram_tensor("vw1T", [FDIM, SIDE], BF, kind="ExternalInput")
    uw2T = nc.dram_tensor("uw2T", [SIDE, OUT], BF, kind="ExternalInput")
    vw2T = nc.dram_tensor("vw2T", [SIDE, OUT], BF, kind="ExternalInput")
    ub1 = nc.dram_tensor("ub1", [SIDE, 1], FP, kind="ExternalInput")
    vb1 = nc.dram_tensor("vb1", [SIDE, 1], FP, kind="ExternalInput")
    sv = nc.dram_tensor("sv", [1, QV], FP, kind="ExternalInput")
    su = nc.dram_tensor("su", [1, QU], FP, kind="ExternalInput")
    u_outT = nc.dram_tensor("u_outT", [OUT, QU], FP, kind="ExternalOutput")
    v_outT = nc.dram_tensor("v_outT", [OUT, QV], FP, kind="ExternalOutput")

    NQ = 4  # DMA/mask quartering of the contraction dim (4 kps per quarter)
    QW = KP * 2 * 512 // NQ  # 8192 free elems per quarter

    with tile.TileContext(nc) as tc, ExitStack() as ctx:
        res = ctx.enter_context(tc.tile_pool(name="res", bufs=1))
        scr = ctx.enter_context(tc.tile_pool(name="scr", bufs=2))

        # ---- input loads (sync queue). Order = need order. ----
        sm = []

        def load(dram_t, shape, dtype, tag, src=None):
            t = res.tile(shape, dtype, tag=tag, name="t")
            nc.sync.dma_start(out=t[:], in_=src if src is not None else dram_t[:, :])
            return t

        # shipped first mask (quartered) + first weights FIRST: they alone
        # gate the start of the MM stream (range-based deps let kp0 run
        # after the first quarter lands)
        im0_sb = scr.tile([128, 16, 512], F8, tag="imask", bufs=3, name="im0")
        nc.sync.dma_start(out=im0_sb[:, 0:4, :], in_=imask0[:, 0:2048])
        wu_sb, wi_sb = [], []
        for kp in range(2):
            t = res.tile([128, 2, R, M], F8, tag=f"wu{kp}", name="t")
            nc.sync.dma_start(out=t[:], in_=wu8[kp])
            wu_sb.append(t)
        for q in range(1, 4):
            nc.sync.dma_start(out=im0_sb[:, q * 4 : (q + 1) * 4, :],
                              in_=imask0[:, q * 2048 : (q + 1) * 2048])
        adjc_sb = res.tile([128, KP * 2, 512], I8, tag="adjc")
        nc.sync.dma_start(out=adjc_sb[:, 0:16, :], in_=adjc[:, 0 : 2 * QW])
        for kp in range(2, 8):
            t = res.tile([128, 2, R, M], F8, tag=f"wu{kp}", name="t")
            nc.sync.dma_start(out=t[:], in_=wu8[kp])
            wu_sb.append(t)
        nc.sync.dma_start(out=adjc_sb[:, 16:32, :], in_=adjc[:, 2 * QW : 4 * QW])
        for kp in range(8, KP):
            t = res.tile([128, 2, R, M], F8, tag=f"wu{kp}", name="t")
            nc.sync.dma_start(out=t[:], in_=wu8[kp])
            wu_sb.append(t)

        vfq_sb = load(vfTq, [FDIM, QV], BF, "vfq")
        ufq_sb = load(ufTq, [FDIM, QU], BF, "ufq")
        sv_sb = load(sv, [1, QV], FP, "svt")
        su_sb = load(su, [1, QU], FP, "sut")
        vw1_sb = load(vw1T, [FDIM, SIDE], BF, "vw1")
        uw1_sb = load(uw1T, [FDIM, SIDE], BF, "uw1")
        vw2_sb = load(vw2T, [SIDE, OUT], BF, "vw2")
        uw2_sb = load(uw2T, [SIDE, OUT], BF, "uw2")
        vb_sb = load(vb1, [SIDE, 1], FP, "vb")
        ub_sb = load(ub1, [SIDE, 1], FP, "ub")
        dwT_sb = []
        for mh in range(2):
            t = res.tile([128, OUT], BF, tag=f"dwT{mh}", name="t")
            nc.sync.dma_start(out=t[:], in_=dwT[mh])
            dwT_sb.append(t)

        for kp in range(KP):
            t = res.tile([128, 2, R, M], F8, tag=f"wi{kp}", name="t")
            nc.sync.dma_start(out=t[:], in_=wi8[kp])
            wi_sb.append(t)

        ps = ctx.enter_context(tc.tile_pool(name="ps", bufs=8, space="PSUM"))

        def bank(nm):
            return ps.tile([128, 512], FP, tag="ps", bufs=8, name=nm)

        Pv = [bank("Pv") for _ in range(2)]
        Pu = [bank("Pu") for _ in range(2)]

        # ---- pass 2 (from PSUM); V is emitted before the last user block ----
        def pass2(P, scale_b, f_q, w2_sb, q, o_dram, nm):
            hvs = []
            for mh in range(2):
                hr = scr.tile([128, 512], BF, tag="p2r", bufs=4, name="hr")
                nc.scalar.activation(out=hr[:, :q], in_=P[mh][:, :q], func=AF.Relu)
                hs = scr.tile([128, 512], BF, tag="p2s", bufs=4, name="hs")
                nc.vector.tensor_tensor(
                    out=hs[:, :q], in0=hr[:, :q], in1=scale_b[:, :q], op=ALU.mult
                )
                hvs.append(hs)
            po = bank(f"po{nm}")
            for mh in range(2):
                nc.tensor.matmul(po[:OUT, :q], lhsT=dwT_sb[mh][:128, :OUT],
                                 rhs=hvs[mh][:, :q], start=(mh == 0), stop=False)
            nc.tensor.matmul(po[:OUT, :q], lhsT=w2_sb[:SIDE, :OUT], rhs=f_q[:, :q],
                             start=False, stop=True)
            vout = scr.tile([OUT, 512], FP, tag="p2o", bufs=2, name="vout")
            nc.scalar.activation(out=vout[:, :q], in_=po[:OUT, :q], func=AF.Relu)
            nc.scalar.dma_start(out=o_dram[:, :], in_=vout[:, :q])

        def emit_pass2_v():
            pass2(Pv, svb, fvq, vw2_sb, QV, v_outT, "v")

        # ---- main MM stream: kp-half-major (DMA pacing), r-major inside ----
        def half_masks(src_sb, h, rv, tag):
            m = scr.tile([128, 16, 512], F8, tag=tag, bufs=3, name="m")
            nc.vector.tensor_scalar(
                out=m[:, :, :], in0=src_sb[:, h * 16 : (h + 1) * 16, :],
                scalar1=rv, scalar2=None, op0=ALU.is_equal,
            )
            return m

        def mm_half(P, w_sb, msk, h, ri, first, last):
            for k in range(8):
                kp = h * 8 + k
                for mh in range(2):
                    nc.tensor.matmul(
                        P[mh][:],
                        lhsT=w_sb[kp][:, :, ri, mh * 128 : (mh + 1) * 128],
                        rhs=msk[:, k * 2 : k * 2 + 2, :],
                        start=(first and k == 0), stop=(last and k == 7),
                        perf_mode=PM.DoubleRow,
                    )

        prep_done = False
        for h in range(2):  # item side, half-major
            for ri in range(R):
                if h == 0 and ri == 0:
                    imask = im0_sb
                else:
                    imask = half_masks(adjc_sb, h, float(ri + 1), "imask")
                mm_half(Pv, wu_sb, imask, h, ri,
                        first=(h == 0 and ri == 0), last=(h == 1 and ri == R - 1))
                if not prep_done:
                    prep_done = True
                    pf = bank("pf")
                    nc.tensor.matmul(pf[:SIDE, :QV], lhsT=vw1_sb[:FDIM, :SIDE],
                                     rhs=vfq_sb[:FDIM, :], start=True, stop=True)
                    fvq = res.tile([SIDE, QV], BF, tag="fvq")
                    nc.scalar.activation(out=fvq[:], in_=pf[:SIDE, :QV],
                                         func=AF.Relu, bias=vb_sb[:, :])
                    pf2 = bank("pf2")
                    nc.tensor.matmul(pf2[:SIDE, :QU], lhsT=uw1_sb[:FDIM, :SIDE],
                                     rhs=ufq_sb[:FDIM, :], start=True, stop=True)
                    fuq = res.tile([SIDE, QU], BF, tag="fuq")
                    nc.scalar.activation(out=fuq[:], in_=pf2[:SIDE, :QU],
                                         func=AF.Relu, bias=ub_sb[:, :])
                    svb = res.tile([128, QV], FP, tag="svb")
                    nc.gpsimd.partition_broadcast(svb[:], sv_sb[:1, :])
                    sub = res.tile([128, QU], FP, tag="sub")
                    nc.gpsimd.partition_broadcast(sub[:], su_sb[:1, :])
        for h in range(2):  # user side
            if h == 1:
                emit_pass2_v()
            for ri in range(R):
                if ri < R_DVE_USER:
                    umask = half_masks(adjt_sb, h, float(ri + 1), "umask")
                else:
                    umask = scr.tile([128, 16, 512], F8, tag="umask", bufs=3,
                                     name="umask")
                    for q in range(2):
                        nc.sync.dma_start(
                            out=umask[:, q * 8 : (q + 1) * 8, :],
                            in_=umask8[ri - R_DVE_USER, :,
                                       h * 2 * QW + q * QW : h * 2 * QW + (q + 1) * QW],
                        )
                mm_half(Pu, wi_sb, umask, h, ri,
                        first=(h == 0 and ri == 0), last=(h == 1 and ri == R - 1))

        pass2(Pu, sub, fuq, uw2_sb, QU, u_outT, "u")

    nc.compile()
    return nc


_CACHE = {}


def _get_program():
    if "nc" not in _CACHE:
        _CACHE["nc"] = build_program()
    return _CACHE["nc"]


def _fp8(x):
    return np.clip(x, -240.0, 240.0).astype(ml_dtypes.float8_e4m3)


def _contraction_layout(arr):
    """[NP, 512] -> [128, KP*2*512] with index (p, kp, s, c), n = kp*256+s*128+p."""
    return np.ascontiguousarray(
        arr.reshape(KP, 2, 128, 512).transpose(2, 0, 1, 3).reshape(128, KP * 2 * 512)
    )


def make_in_maps(inputs):
    adj = np.asarray(inputs["adj_matrix"], dtype=np.int32)
    msg_W = np.asarray(inputs["msg_W"], dtype=np.float32)
    u_sf = np.asarray(inputs["u_sideFeat"], dtype=np.float32)
    v_sf = np.asarray(inputs["v_sideFeat"], dtype=np.float32)
    dense_W = np.asarray(inputs["dense_W"], dtype=np.float32)
    u_W1 = np.asarray(inputs["u_W1"], dtype=np.float32)
    u_b1 = np.asarray(inputs["u_b1"], dtype=np.float32).reshape(SIDE, 1)
    u_W2 = np.asarray(inputs["u_W2"], dtype=np.float32)
    v_W1 = np.asarray(inputs["v_W1"], dtype=np.float32)
    v_b1 = np.asarray(inputs["v_b1"], dtype=np.float32).reshape(SIDE, 1)
    v_W2 = np.asarray(inputs["v_W2"], dtype=np.float32)

    nz = adj != 0
    cu = 1.0 / np.sqrt(np.maximum(nz.sum(1), 1).astype(np.float32))
    cv = 1.0 / np.sqrt(np.maximum(nz.sum(0), 1).astype(np.float32))
    Wu = msg_W[:, :, :NU]
    Wi = msg_W[:, :, NU:]

    adj_pad = np.zeros((NP, NP), np.int8)
    adj_pad[:NU, :NI] = adj.astype(np.int8)

    # replicated fp8 weights over the FULL contraction dim (same for all cores)
    wus = np.zeros((R, NP, M), np.float32)
    wus[:, :NU, :] = ALPHA * cu[None, :, None] * Wu.transpose(0, 2, 1)
    wu8_h = np.ascontiguousarray(
        _fp8(wus).reshape(R, KP, 2, 128, M).transpose(1, 3, 2, 0, 4)
    )
    wis = np.zeros((R, NP, M), np.float32)
    wis[:, :NI, :] = ALPHA * cv[None, :, None] * Wi.transpose(0, 2, 1)
    wi8_h = np.ascontiguousarray(
        _fp8(wis).reshape(R, KP, 2, 128, M).transpose(1, 3, 2, 0, 4)
    )

    dwT_h = np.ascontiguousarray(
        dense_W.T.reshape(2, 128, OUT).astype(ml_dtypes.bfloat16)
    )
    uw1T_h = np.ascontiguousarray(u_W1.T.astype(ml_dtypes.bfloat16))
    vw1T_h = np.ascontiguousarray(v_W1.T.astype(ml_dtypes.bfloat16))
    uw2T_h = np.ascontiguousarray(u_W2.T.astype(ml_dtypes.bfloat16))
    vw2T_h = np.ascontiguousarray(v_W2.T.astype(ml_dtypes.bfloat16))

    in_maps = []
    for a in range(GA):
        for b in range(GB):
            vi0 = b * BI + a * QV  # first owned item (global)
            ui0 = a * BU + b * QU  # first owned user (global)
            vn = max(0, min(QV, NI - vi0))
            un = max(0, min(QU, NU - ui0))

            acols = np.zeros((NP, QV), np.int8)
            acols[:, :vn] = adj_pad[:, vi0 : vi0 + vn]
            adjc_h = _contraction_layout(acols)
            atcols = np.zeros((NP, QU), np.int8)
            atcols[:, :un] = adj_pad.T[:, ui0 : ui0 + un]
            adjt_h = _contraction_layout(atcols)
            um = np.empty((R_SHIP, 128, KP * 2 * 512), ml_dtypes.float8_e4m3)
            for j in range(R_SHIP):
                oh = (atcols == (R_DVE_USER + 1 + j)).astype(ml_dtypes.float8_e4m3)
                um[j] = _contraction_layout(oh)

            im0_h = np.ascontiguousarray(
                _contraction_layout(
                    (acols == 1).astype(ml_dtypes.float8_e4m3)
                )[:, : 2 * (KP * 2 * 512) // 4]
            )
            vfq_h = np.zeros((FDIM, QV), ml_dtypes.bfloat16)
            vfq_h[:, :vn] = v_sf[vi0 : vi0 + vn].T.astype(ml_dtypes.bfloat16)
            sv_h = np.zeros((1, QV), np.float32)
            sv_h[0, :vn] = cv[vi0 : vi0 + vn] / ALPHA
            ufq_h = np.zeros((FDIM, QU), ml_dtypes.bfloat16)
            ufq_h[:, :un] = u_sf[ui0 : ui0 + un].T.astype(ml_dtypes.bfloat16)
            su_h = np.zeros((1, QU), np.float32)
            su_h[0, :un] = cu[ui0 : ui0 + un] / ALPHA

            in_maps.append(
                {
                    "adjc": adjc_h,
                    "imask0": im0_h,
                    "adjt": adjt_h,
                    "umask8": np.ascontiguousarray(um),
                    "wu8": wu8_h,
                    "wi8": wi8_h,
                    "vfTq": vfq_h,
                    "ufTq": ufq_h,
                    "dwT": dwT_h,
                    "uw1T": uw1T_h,
                    "vw1T": vw1T_h,
                    "uw2T": uw2T_h,
                    "vw2T": vw2T_h,
                    "ub1": u_b1,
                    "vb1": v_b1,
                    "sv": sv_h,
                    "su": su_h,
                }
            )
    return in_maps


def assemble(results):
    U = np.empty((NU, OUT), np.float32)
    V = np.empty((NI, OUT), np.float32)
    for a in range(GA):
        for b in range(GB):
            cid = a * GB + b
            vi0 = b * BI + a * QV
            ui0 = a * BU + b * QU
            vn = max(0, min(QV, NI - vi0))
            un = max(0, min(QU, NU - ui0))
            U[ui0 : ui0 + un] = results[cid]["u_outT"].T[:un]
            V[vi0 : vi0 + vn] = results[cid]["v_outT"].T[:vn]
    return (U, V)


def kernel(**inputs):
    from concourse.bass_utils import run_bass_kernel_spmd

    nc = _get_program()
    res = run_bass_kernel_spmd(nc, make_in_maps(inputs), core_ids=list(range(NCORES)))
    return assemble(res.results)
